# revision 4
# baseline (speedup 1.0000x reference)
"""KernelConv for Trainium2: out[c,h,w] = sum_t softmax_t(core[t,c,h,w]) * frames[c,h+di,w+dj].

Sharding: 2(H) x 4(W) grid over 8 NeuronCores; each core gets a contiguous
[147, 360, 320] slice of core plus a halo-padded [3, 366, 326] frames slice,
so no device-to-device exchange is needed.

Per-core pipeline (3 row-blocks of 120 rows):
  DMA 7-tap core chunks (f32) -> ScalarE exp -> bf16
  VectorE: e * shifted-frame view (bf16, 2x mode)
  TensorE: identity-matmul accumulation of products and of e into PSUM (f32)
  VectorE: reciprocal + multiply, DMA out (f32)
"""

import numpy as np
import ml_dtypes

import concourse.bass as bass
import concourse.tile as tile
import concourse.mybir as mybir
from concourse.bass_utils import run_bass_kernel_spmd
from concourse.masks import make_identity

C, H, W = 3, 720, 1280
K = 7
PAD = K // 2
NT = K * K  # 49 taps
HSH, WSH = 2, 4  # shard grid
DH, DW = H // HSH, W // WSH  # 360 x 320 per device
RB = 120  # row-block
NRB = DH // RB
FH, FW = DH + 2 * PAD, DW + 2 * PAD  # 366 x 326 frames slice w/ halo
G = 7  # taps per DMA/ACT group
NG = NT // G
FREE = C * DW  # 960

_cached = {}


def make_nop(nc, engine, waits):
    inst = nc.engines[engine].nop(hint="waitsplit", nofuse=True).ins
    for bb in nc.main_func.blocks:
        if inst in bb.instructions:
            bb.instructions.remove(inst)
            break
    inst.sync_info = mybir.SyncInfo(on_wait=list(waits), on_update=[])
    return inst


def legalize_sync_waits(nc, cap=1):
    # this walrus build accepts at most one sync-wait per instruction; hoist
    # the rest onto same-engine NOPs placed immediately before
    for bb in nc.main_func.blocks:
        out = []
        changed = False
        for inst in list(bb.instructions):
            si = inst.sync_info
            waits = list(si.on_wait) if si and si.on_wait else []
            if len(waits) > cap:
                keep = waits[-cap:]
                extra = waits[: len(waits) - cap]
                for i in range(0, len(extra), cap):
                    out.append(make_nop(nc, inst.engine, extra[i : i + cap]))
                inst.sync_info = mybir.SyncInfo(
                    on_wait=keep, on_update=list(si.on_update) if si.on_update else []
                )
                changed = True
            out.append(inst)
        if changed:
            bb.instructions = out


def build_module():
    nc = bass.Bass("TRN2", target_bir_lowering=False, debug=False, num_devices=1)
    f32, bf16 = mybir.dt.float32, mybir.dt.bfloat16
    core_d = nc.dram_tensor("core_s", [NT * C, DH, DW], f32, kind="ExternalInput")
    fp_d = nc.dram_tensor("fp_s", [C, FH, FW], bf16, kind="ExternalInput")
    out_d = nc.dram_tensor("out_s", [C, DH, DW], f32, kind="ExternalOutput")

    core_v = core_d.ap().rearrange("(t c) h w -> h t c w", c=C)  # [360,49,3,320]
    fp_v = fp_d.ap().rearrange("c h w -> h c w")  # [366,3,326]
    out_v = out_d.ap().rearrange("c h w -> h c w")  # [360,3,320]

    with tile.TileContext(nc) as tc:
        with (
            tc.tile_pool(name="singles", bufs=1) as singles,
            tc.tile_pool(name="cpool", bufs=2) as cpool,
            tc.tile_pool(name="epool", bufs=2) as epool,
            tc.tile_pool(name="ppool", bufs=4) as ppool,
            tc.tile_pool(name="fpool", bufs=2) as fpool,
            tc.tile_pool(name="opool", bufs=2) as opool,
            tc.tile_pool(name="psum", bufs=2, space="PSUM") as psum,
        ):
            idn = singles.tile([RB, RB], bf16)
            make_identity(nc, idn[:])

            for rb in range(NRB):
                r0 = rb * RB
                # all 7 row shifts in one tile: compute ops must start at
                # partition 0, so the row shift lives in a free dim instead
                ft = fpool.tile([RB, K, C, FW], bf16, tag="ft")
                fpap = fp_d.ap()
                for c in range(C):
                    nc.sync.dma_start(
                        out=ft[:, :, c, :],
                        in_=bass.AP(
                            tensor=fpap.tensor,
                            offset=c * FH * FW + r0 * FW,
                            ap=[[FW, RB], [FW, K], [1, FW]],
                        ),
                    )
                fto = fpool.tile([RB, K, C, FW], bf16, tag="fto")
                # odd-w-shift copy so odd-j taps keep 4B alignment (2x mode)
                nc.vector.tensor_copy(fto[:, :, :, 0 : FW - 1], ft[:, :, :, 1:FW])

                acc = psum.tile([RB, FREE], mybir.dt.float32, tag="acc")
                se = psum.tile([RB, FREE], mybir.dt.float32, tag="se")

                for g in range(NG):
                    ct = cpool.tile([RB, G, C, DW], mybir.dt.float32, tag="ct")
                    nc.sync.dma_start(
                        out=ct[:], in_=core_v[r0 : r0 + RB, g * G : (g + 1) * G]
                    )
                    et = epool.tile([RB, G, C, DW], bf16, tag="et")
                    nc.scalar.activation(et[:], ct[:], mybir.ActivationFunctionType.Exp)
                    et_flat = et[:].rearrange("p g c w -> p (g c w)")
                    for k in range(G):
                        t = g * G + k
                        i, j = t // K, t % K
                        if j % 2 == 0:
                            fv = ft[:, i, :, j : j + DW]
                        else:
                            fv = fto[:, i, :, j - 1 : j - 1 + DW]
                        pt = ppool.tile([RB, FREE], bf16, tag="pt")
                        nc.vector.tensor_mul(
                            pt[:].rearrange("p (c w) -> p c w", c=C), et[:, k], fv
                        )
                        first, last = t == 0, t == NT - 1
                        ek = et_flat[:, k * FREE : (k + 1) * FREE]
                        for lo, hi in ((0, 512), (512, FREE)):
                            nc.tensor.matmul(
                                acc[:, lo:hi], idn[:], pt[:, lo:hi],
                                start=first, stop=last, skip_group_check=True,
                            )
                            nc.tensor.matmul(
                                se[:, lo:hi], idn[:], ek[:, lo:hi],
                                start=first, stop=last, skip_group_check=True,
                            )

                rcp = opool.tile([RB, FREE], mybir.dt.float32, tag="rcp")
                nc.vector.reciprocal(rcp[:], se[:])
                ot = opool.tile([RB, FREE], mybir.dt.float32, tag="ot")
                nc.vector.tensor_mul(ot[:], acc[:], rcp[:])
                nc.sync.dma_start(
                    out=out_v[r0 : r0 + RB],
                    in_=ot[:].rearrange("p (c w) -> p c w", c=C),
                )

    legalize_sync_waits(nc)
    return nc


def _shard_inputs(frames, core):
    fr = np.asarray(frames, np.float32).reshape(C, H, W)
    co = np.asarray(core, np.float32).reshape(NT * C, H, W)
    fp = np.zeros((C, H + 2 * PAD, W + 2 * PAD), np.float32)
    fp[:, PAD : PAD + H, PAD : PAD + W] = fr
    fp16 = fp.astype(ml_dtypes.bfloat16)
    in_maps = []
    for hs in range(HSH):
        for ws in range(WSH):
            cs = np.ascontiguousarray(
                co[:, hs * DH : (hs + 1) * DH, ws * DW : (ws + 1) * DW]
            )
            fs = np.ascontiguousarray(
                fp16[:, hs * DH : hs * DH + FH, ws * DW : ws * DW + FW]
            )
            in_maps.append({"core_s": cs, "fp_s": fs})
    return in_maps


def kernel(frames, core):
    if "nc" not in _cached:
        _cached["nc"] = build_module()
    nc = _cached["nc"]
    in_maps = _shard_inputs(frames, core)
    res = run_bass_kernel_spmd(nc, in_maps, core_ids=list(range(HSH * WSH)))
    out = np.empty((1, C, H, W), np.float32)
    d = 0
    for hs in range(HSH):
        for ws in range(WSH):
            out[0, :, hs * DH : (hs + 1) * DH, ws * DW : (ws + 1) * DW] = res.results[
                d
            ]["out_s"]
            d += 1
    return out
